# revision 2
# baseline (speedup 1.0000x reference)
"""CapsNet dynamic-routing kernel for 8 Trainium2 NeuronCores — v2.

Route-sharded (512 routes/core). All data fp16 on-chip (tolerance 2e-2;
fp16 keeps elementwise error ~0.05%):
  - production: per-g matmuls, stationary = block-diag x (fp16), moving =
    host-pretransposed W (fp16, layout (o,c) so every later DVE op has a
    unit-stride 16-bit innermost dim -> 2x_1P packed mode)
  - u_hat resident in SBUF as [p=(rb,b), (g,o,c)] fp16 (64 KB/partition)
  - routing sweeps: a-pass = DVE TT (u*v_rep) + o-reduction split between
    DVE tree-adds and Pool tensor_reduce; batch-mean via PE delta matmul;
    e-update is multiplicative (e *= exp(a_mean)) so b_ij never
    materializes; s-pass = DVE TT (e*u) + per-g PE delta matmuls
  - collectives: AllGather (cheaper than AllReduce) of the [16,544] fp16
    partial route-sums; the 8-way sum after the gather is one PE delta
    matmul over the gathered [128,544] tile
"""

import numpy as np

B, R, I, C, O = 16, 4096, 16, 32, 16
NCORES = 8
RL = R // NCORES      # 512 routes per core
G = RL // 8           # 64 groups of 8 routes
CO = C * O            # 512
CHG = 4               # groups per DVE chunk
NCH = G // CHG        # 16 chunks
NBLK = 4              # e-update blocks (16 g each)

_cache = {}


def _patch_tile_drain():
    import concourse.tile as tile_mod
    from concourse.vector_clock import ScopedClock, VectorClock

    if getattr(tile_mod.TileContext, "_drain_patched", False):
        return

    def _split_drain_and_barrier(self, tick_clock, wait_clock):
        ticks = list(tick_clock.global_clock)
        for i in [j for j, t in enumerate(ticks) if t > 0]:
            vec = [ticks[j] if j == i else 0 for j in range(len(ticks))]
            d = self.nc.sync.drain()
            wait_clock.add_sem_waits(d.ins, ScopedClock({None: VectorClock(vec)}))
        self.nc.all_engine_barrier()
        popped = self.nc._tile_sem_poison_stack.pop()
        assert popped is self._sem_poison
        self.nc.clear_and_free_semaphores(list(self.sems.allocated().values()))
        self.nc.all_engine_barrier()

    tile_mod.TileContext._drain_and_barrier = _split_drain_and_barrier
    tile_mod.TileContext._drain_patched = True


def _split_waits(nc, limit=1):
    """This container's walrus rejects >1 sync-wait per instruction; move
    excess waits onto same-engine NoOps inserted just before the owner."""
    import concourse.mybir as mybir

    blocks = nc.main_func.blocks
    for bb in blocks:
        insts = bb.instructions  # live list view
        k = 0
        while k < len(insts):
            inst = insts[k]
            si = inst.sync_info
            if si is not None and si.on_wait and len(si.on_wait) > limit:
                w = list(si.on_wait)
                si.on_wait = w[:limit]
                excess = w[limit:]
                insert_at = k
                for cs in range(0, len(excess), limit):
                    chunk = excess[cs:cs + limit]
                    nop = nc.engines[inst.engine].nop()
                    ni = nop.ins
                    for bb2 in blocks:
                        l2 = bb2.instructions
                        hit = next(
                            (i for i in range(len(l2) - 1, -1, -1)
                             if l2[i].name == ni.name), None)
                        if hit is not None:
                            l2.pop(hit)
                            break
                    ni.sync_info = mybir.SyncInfo(on_wait=chunk, on_update=[])
                    insts.insert(insert_at, ni)
                    insert_at += 1
                    k += 1
            k += 1


def _build_nc():
    import concourse.bass as bass
    import concourse.mybir as mybir
    from concourse.tile import TileContext

    _patch_tile_drain()
    F32 = mybir.dt.float32
    F16 = mybir.dt.float16
    AF = mybir.ActivationFunctionType
    ALU = mybir.AluOpType
    CORES = list(range(NCORES))

    nc = bass.Bass(target_bir_lowering=False)
    # wt[blk, 128=(rb,i), 16g, 512=(o,c)] fp16
    wt_d = nc.declare_dram_parameter("wt", [4, 128, 16, 512], F16, isOutput=False)
    # xb2[blk, 128=(rb,i), 8pair, 256=(half,(rb,b))] fp16 block-diag x
    xb_d = nc.declare_dram_parameter("xb", [4, 128, 8, 256], F16, isOutput=False)
    db_d = nc.declare_dram_parameter("delta_b", [128, 16], F16, isOutput=False)
    dbs_d = nc.declare_dram_parameter("delta_bs", [128, 16], F16, isOutput=False)
    ob_d = nc.declare_dram_parameter("ones_bd", [128, 128], F16, isOutput=False)
    o16_d = nc.declare_dram_parameter("ones_16", [128, 16], F16, isOutput=False)
    dr_d = nc.declare_dram_parameter("delta_rep", [16, 128], F16, isOutput=False)
    out_d = nc.declare_dram_parameter("out", [16, CO], F32, isOutput=True)
    cc_in = [nc.dram_tensor(f"cc_in{k}", [16, 544], F16) for k in range(3)]
    cc_out = [
        nc.dram_tensor(f"cc_out{k}", [128, 544], F16, addr_space="Shared")
        for k in range(3)
    ]

    with TileContext(nc) as tc:
        with (
            nc.allow_low_precision(reason="fp16 kernel; tolerance 2e-2"),
            tc.tile_pool(name="big", bufs=1) as big,
            tc.tile_pool(name="stw", bufs=2) as stw,
            tc.tile_pool(name="stx", bufs=4) as stx,
            tc.tile_pool(name="work", bufs=3) as work,
            tc.tile_pool(name="wa", bufs=6) as wa,
            tc.tile_pool(name="ws", bufs=4) as ws,
            tc.tile_pool(name="small", bufs=1) as small,
            tc.tile_pool(name="psum", bufs=1, space="PSUM") as psum,
            tc.tile_pool(name="psum_u", bufs=3, space="PSUM") as psum_u,
        ):
            # constants
            db = small.tile([128, 16], F16, tag="db")
            dbs = small.tile([128, 16], F16, tag="dbs")
            ob = small.tile([128, 128], F16, tag="ob")
            o16 = small.tile([128, 16], F16, tag="o16")
            dr = small.tile([16, 128], F16, tag="dr")
            # consts ride the Pool SWDGE queue so SP dispatches the big wt
            # DMAs without queueing behind them
            nc.gpsimd.dma_start(out=db[:, :], in_=db_d[:, :])
            nc.gpsimd.dma_start(out=dbs[:, :], in_=dbs_d[:, :])
            nc.gpsimd.dma_start(out=ob[:, :], in_=ob_d[:, :])
            nc.gpsimd.dma_start(out=o16[:, :], in_=o16_d[:, :])
            nc.gpsimd.dma_start(out=dr[:, :], in_=dr_d[:, :])

            # u_hat resident fp16: [p=(rb,b), g, o, c]
            u_sb = big.tile([128, G, O, C], F16, tag="u")
            q = small.tile([128, G, C], F16, tag="q")
            e_rep = small.tile([128, G, C], F16, tag="e_rep")
            v_rep = small.tile([128, O, C], F16, tag="v_rep")
            st = small.tile([16, 544], F16, tag="st")
            s_t = small.tile([16, O, C], F16, tag="s")
            sq = small.tile([16, CO], F16, tag="sq")
            ab = small.tile([16, CO], F16, tag="ab")
            num = small.tile([16, CO], F16, tag="num")
            den = small.tile([16, CO], F16, tag="den")
            v_t = small.tile([16, CO], F16, tag="v")
            v_f32 = small.tile([16, CO], F32, tag="vf")
            rdn = small.tile([16, C], F32, tag="rdn")
            nc.vector.memset(st[:, 512:], 0.0)
            nln256 = small.tile([128, 1], F32, tag="nln256")
            nc.vector.memset(nln256[:, :], -5.545177444479562)  # -ln(256)

            drain_engines = [nc.vector, nc.scalar]

            # ---- production + iteration-0 route-sum ----
            # s0MM runs one group behind pu so PE never waits on a drain
            ps_s0 = psum.tile([16, CO], F32, tag="acc")
            s0_pending = []
            for blk in range(4):
                wt_t = stw.tile([128, 16, 512], F16, tag="wt")
                xb_t = stx.tile([128, 8, 256], F16, tag="xb")
                # xb via the Pool queue so SP only dispatches the big wt DMAs
                # (SP serializes whole DMA timelines; splitting queues keeps
                # the DMA engines fed)
                nc.gpsimd.dma_start(out=xb_t[:, :, :], in_=xb_d[blk, :, :, :])
                nc.sync.dma_start(out=wt_t[:, :, :], in_=wt_d[blk, :, :, :])
                for k in range(16):
                    g = blk * 16 + k
                    pu = psum_u.tile([128, 512], F32, tag="pu")
                    nc.tensor.matmul(
                        pu[:, :],
                        xb_t[:, k // 2, (k % 2) * 128:(k % 2) * 128 + 128],
                        wt_t[:, k, :],
                        start=True, stop=True,
                    )
                    # drain split across DVE+ACT so the PSUM ring frees at
                    # PE's matmul rate
                    puv = pu[:, :].rearrange("p (o c) -> p o c", o=O)
                    nc.vector.tensor_copy(u_sb[:, g, 0:8, :], puv[:, 0:8, :])
                    nc.scalar.copy(u_sb[:, g, 8:16, :], puv[:, 8:16, :])
                    # s0 += (1/R) * sum_rb u   (dbs folds the 1/R)
                    s0_pending.append(g)
                    if len(s0_pending) > 1:
                        gp = s0_pending.pop(0)
                        nc.tensor.matmul(
                            ps_s0[:, :], dbs[:, :],
                            u_sb[:, gp, :, :].rearrange("p o c -> p (o c)"),
                            start=(gp == 0), stop=False,
                        )

            def all_gather_sum(it, with_dn):
                """st [16,544] f16 -> gathered sum in two PSUM tiles.
                Returns (ps_sg [16,512] f32, ps_dng [16,32] f32 or None)."""
                nc.sync.dma_start(out=cc_in[it][:, :], in_=st[:, :])
                nc.gpsimd.collective_compute(
                    "AllGather", ALU.bypass, replica_groups=[CORES],
                    ins=[cc_in[it][:, :]], outs=[cc_out[it][:, :]],
                )
                gat = work.tile([128, 544], F16, tag="gat")
                nc.sync.dma_start(out=gat[:, :], in_=cc_out[it][:, :])
                ps_sg = psum.tile([16, CO], F32, tag="sg")
                nc.tensor.matmul(
                    ps_sg[:, :], db[:, :], gat[:, :512], start=True, stop=True,
                )
                ps_dng = None
                if with_dn:
                    ps_dng = psum.tile([16, C], F32, tag="dng")
                    nc.tensor.matmul(
                        ps_dng[:, :], db[:, :], gat[:, 512:544],
                        start=True, stop=True,
                    )
                return ps_sg, ps_dng

            def squash(ps_sg, ps_dng, out_f32=False):
                """v = squash(s); s = ps_sg (/dn if ps_dng). All [16, (o,c)]."""
                if ps_dng is None:
                    nc.vector.tensor_copy(s_t[:, :, :],
                                          ps_sg[:, :].rearrange("p (o c) -> p o c", o=O))
                else:
                    nc.vector.reciprocal(rdn[:, :], ps_dng[:, :])
                    nc.vector.tensor_tensor(
                        s_t[:, :, :],
                        ps_sg[:, :].rearrange("p (o c) -> p o c", o=O),
                        rdn[:, :].unsqueeze(1).broadcast_to([16, O, C]),
                        ALU.mult,
                    )
                sf = s_t[:, :, :].rearrange("p o c -> p (o c)")
                # v = s*|s| / (1+s^2)
                nc.vector.tensor_tensor(sq[:, :], sf, sf, ALU.mult)
                nc.scalar.activation(ab[:, :], sf, AF.Abs)
                nc.vector.tensor_tensor(num[:, :], sf, ab[:, :], ALU.mult)
                nc.vector.tensor_scalar_add(den[:, :], sq[:, :], 1.0)
                nc.vector.reciprocal(den[:, :], den[:, :])
                tgt = v_f32 if out_f32 else v_t
                nc.vector.tensor_tensor(tgt[:, :], num[:, :], den[:, :], ALU.mult)

            gp = s0_pending.pop(0)
            nc.tensor.matmul(
                ps_s0[:, :], dbs[:, :],
                u_sb[:, gp, :, :].rearrange("p o c -> p (o c)"),
                start=False, stop=True,
            )

            # ---- iteration 0: uniform c_ij ----
            nc.scalar.copy(st[:, :512], ps_s0[:, :])
            ps_sg, _ = all_gather_sum(0, with_dn=False)
            squash(ps_sg, None)

            # ---- routing sweeps (iterations 1, 2) ----
            for it in (1, 2):
                # v_rep[(rb,b), (o,c)] = v[b, (o,c)]
                ps_vr = psum.tile([128, CO], F32, tag="ps_vr")
                nc.tensor.matmul(ps_vr[:, :], dr[:, :], v_t[:, :],
                                 start=True, stop=True)
                nc.scalar.copy(v_rep[:, :, :],
                               ps_vr[:, :].rearrange("p (o c) -> p o c", o=O))

                ps_st = psum.tile([16, CO], F32, tag="acc")

                def a_block(j):
                    # a-pass for block j: prod = u * v_rep (fp16 2x), then
                    # o-reduction tree (most chunks on Pool to unload DVE)
                    for cc in range(NCH // NBLK):
                        ch = j * (NCH // NBLK) + cc
                        gs = ch * CHG
                        prod = wa.tile([128, CHG, O, C], F16, tag="prod")
                        nc.vector.tensor_tensor(
                            prod[:, :, :, :],
                            u_sb[:, gs:gs + CHG, :, :],
                            v_rep[:, :, :].unsqueeze(1)
                            .broadcast_to([128, CHG, O, C]),
                            ALU.mult,
                        )
                        eng = nc.vector if ch in (0, 3, 6, 9, 12) else nc.gpsimd
                        t8 = wa.tile([128, CHG, 8, C], F16, tag="t8")
                        eng.tensor_tensor(
                            t8[:, :, :, :], prod[:, :, 0:8, :],
                            prod[:, :, 8:16, :], ALU.add)
                        eng.tensor_tensor(
                            t8[:, :, 0:4, :], t8[:, :, 0:4, :],
                            t8[:, :, 4:8, :], ALU.add)
                        eng.tensor_tensor(
                            t8[:, :, 0:2, :], t8[:, :, 0:2, :],
                            t8[:, :, 2:4, :], ALU.add)
                        eng.tensor_tensor(
                            q[:, gs:gs + CHG, :].unsqueeze(2),
                            t8[:, :, 0:1, :], t8[:, :, 1:2, :], ALU.add)

                def e_update(j):
                    # batch-mean a for block j, then e *= exp(a_mean)/256
                    # (b_ij never materialized; 1/256 cancels in e/dn and
                    # keeps fp16 in range)
                    bgs = j * 16
                    ps_am = psum_u.tile([128, 512], F32, tag="pu")
                    nc.tensor.matmul(
                        ps_am[:, :], ob[:, :],
                        q[:, bgs:bgs + 16, :].rearrange("p g c -> p (g c)"),
                        start=True, stop=True,
                    )
                    ev = e_rep[:, bgs:bgs + 16, :].rearrange("p g c -> p (g c)")
                    if it == 1:
                        nc.scalar.activation(ev, ps_am[:, :], AF.Exp,
                                             bias=nln256[:, :])
                    else:
                        ex = work.tile([128, 512], F16, tag="ex")
                        nc.scalar.activation(ex[:, :], ps_am[:, :], AF.Exp,
                                             bias=nln256[:, :])
                        nc.vector.tensor_tensor(ev, ev, ex[:, :], ALU.mult)

                def s_block(j):
                    # s-pass for block j: prod2 = e * u, then delta matmuls
                    for cc in range(NCH // NBLK):
                        ch = j * (NCH // NBLK) + cc
                        gs = ch * CHG
                        prod2 = ws.tile([128, CHG, O, C], F16, tag="prod2")
                        nc.vector.tensor_tensor(
                            prod2[:, :, :, :],
                            u_sb[:, gs:gs + CHG, :, :],
                            e_rep[:, gs:gs + CHG, :].unsqueeze(2)
                            .broadcast_to([128, CHG, O, C]),
                            ALU.mult,
                        )
                        for gg in range(CHG):
                            gi = gs + gg
                            nc.tensor.matmul(
                                ps_st[:, :], db[:, :],
                                prod2[:, gg, :, :].rearrange("p o c -> p (o c)"),
                                start=(gi == 0), stop=(gi == G - 1),
                            )

                # software pipeline: a-pass runs one block ahead so the DVE
                # never stalls on Pool trees / the e-update chain
                a_block(0)
                for j in range(NBLK):
                    if j + 1 < NBLK:
                        a_block(j + 1)
                    e_update(j)
                    s_block(j)
                # local softmax denominator: dn[c] = sum_{local r} e
                dn = work.tile([128, C], F16, tag="dn")
                nc.vector.tensor_reduce(
                    dn[:, :], e_rep[:, :, :].transpose([0, 2, 1]),
                    mybir.AxisListType.X, ALU.add,
                )
                ps_dn = psum.tile([16, C], F32, tag="ps_dn")
                nc.tensor.matmul(ps_dn[:, :], o16[:, :], dn[:, :],
                                 start=True, stop=True)
                nc.scalar.copy(st[:, :512], ps_st[:, :])
                nc.scalar.copy(st[:, 512:544], ps_dn[:, :])
                ps_sg, ps_dng = all_gather_sum(it, with_dn=True)
                squash(ps_sg, ps_dng, out_f32=(it == 2))

            # reorder (o,c) -> (c,o) and write out
            vo = small.tile([16, CO], F32, tag="vo")
            nc.vector.tensor_copy(
                vo[:, :].rearrange("p (c o) -> p c o", c=C),
                v_f32[:, :].rearrange("p (o c) -> p o c", o=O).transpose([0, 2, 1]),
            )
            nc.sync.dma_start(out=out_d[:, :], in_=vo[:, :])

    _split_waits(nc)
    return nc


def _prep_inputs(x, W):
    x16 = np.asarray(x, np.float32).astype(np.float16)
    W16 = np.asarray(W, np.float32).astype(np.float16)
    # wt[core, blk, (rb,i), k, (o,c)]
    Wv = W16.reshape(NCORES, G, 8, C, O, I)          # [core,g,rb,c,o,i]
    wt = Wv.transpose(0, 1, 2, 5, 4, 3).reshape(NCORES, 4, 16, 8, I, O * C)
    #    [core, blk, k, rb, i, (o,c)]
    wt = np.ascontiguousarray(
        wt.transpose(0, 1, 3, 4, 2, 5).reshape(NCORES, 4, 128, 16, 512)
    )
    # xb2[core, blk, (rb,i), pair, (half,(rb,b))]
    xv = np.ascontiguousarray(x16.transpose(1, 2, 0)).reshape(
        NCORES, G, 8, I, B)                           # [core,g,rb,i,b]
    xb = np.zeros((NCORES, 32, 8, I, 2, 128), np.float16)
    # xb[core, pair, rb, i, half, rb*16+b] = xv[core, 2*pair+half, rb, i, b]
    for rb in range(8):
        xb[:, :, rb, :, 0, rb * 16:(rb + 1) * 16] = xv[:, 0::2, rb]
        xb[:, :, rb, :, 1, rb * 16:(rb + 1) * 16] = xv[:, 1::2, rb]
    xb = np.ascontiguousarray(
        xb.reshape(NCORES, 32, 128, 256)
        .reshape(NCORES, 4, 8, 128, 256)
        .transpose(0, 1, 3, 2, 4)
    )                                                  # [core, blk, 128, 8, 256]
    db = np.tile(np.eye(16, dtype=np.float16), (8, 1))            # [128,16]
    dbs = (np.tile(np.eye(16, dtype=np.float32), (8, 1)) / R).astype(np.float16)
    ob = np.kron(np.eye(8, dtype=np.float16),
                 np.full((16, 16), 1.0 / B, np.float16))          # [128,128]
    o16 = np.full((128, 16), 1.0 / 16.0, np.float16)
    dr = np.tile(np.eye(16, dtype=np.float16), (1, 8))            # [16,128]
    in_maps = []
    for c in range(NCORES):
        in_maps.append({
            "wt": wt[c], "xb": xb[c],
            "delta_b": db, "delta_bs": dbs, "ones_bd": ob,
            "ones_16": o16, "delta_rep": dr,
        })
    return in_maps


def kernel(x, W):
    from concourse.bass_utils import run_bass_kernel_spmd

    if "nc" not in _cache:
        _cache["nc"] = _build_nc()
    in_maps = _prep_inputs(x, W)
    res = run_bass_kernel_spmd(_cache["nc"], in_maps, list(range(NCORES)))
    v = res.results[0]["out"].reshape(B, C, O)[..., None]
    return np.ascontiguousarray(v, np.float32)
